# revision 44
# baseline (speedup 1.0000x reference)
"""Bridgeout FC layer (dense_mlp) Trainium2 kernel.

out[b, o] = sum_i x[b,i] * (w[i,o] + |w[i,o]| * noise[b,i,o]) + bias[o]

Strategy (8 NeuronCores, contraction-parallel):
  - Each core owns a 128-row slice of the contraction index i. It reads
    noise[:, islice, :] (its 32 MB share of the 256 MB noise tensor) and
    weight[islice, :] (NOT replicated as batch sharding would) and
    produces partial[b, o] = sum_{i in islice} x*(w+|w|*noise); the
    host adds the 8 partials plus the bias.
  - Everything ships as float16 (the 2e-2 rel-err gate leaves >10x
    margin over fp16's 0.05% element error), halving noise DMA bytes:
    16 MB/core at the measured ~420-440 GB/s aggregate DMA rate.
  - Block-diagonal matmuls: the i-slice is split into 8 sub-slices of
    16; the host interleaves noise so SBUF partition j*16+u holds
    sample (g*8+j)'s sub-row u. Then lhsT[128, 8] is a block-diagonal
    x matrix (zero blocks kill cross-sample terms) and ONE matmul
    computes a [8 samples, 512] psum block while streaming 512 f16
    columns at 1 col/cycle -- wide on both M and N. Naive alternatives
    lose: per-sample moving-product matmuls (M=1, N=512) leave 64
    narrow [1,512] psum tiles whose copies serialize ~55 us on the
    scalar engine (matmul psum writes must start at partition 0/32/64,
    so rows can't pack); per-sample stationary-product matmuls (M=128,
    N=1) pay ~170 ns pipeline+LDWEIGHTS cost per single streamed
    column (measured: 79 us kernel, PE-bound).
  - The product pt = |w| (*) noise runs on the DVE in f16 (16-bit
    packed operands hit the 2x mode; measured 2.29 us per [128,4096]
    chunk ~ matching the 2.38 us DMA cadence of 1 MB chunks). |w|
    ships pre-replicated in the interleaved layout, quantized to uint8
    (1 MB; the ~435 GB/s 16-engine DMA aggregate is the kernel's pole,
    so bytes are the only lever) and is dequantized once on the scalar
    engine with a global scale baked into the program.
  - The x@w term seeds each group's psum block with one batched f16
    matmul (lhsT = xT columns) before the noise matmuls accumulate.
  - The whole 16 MB noise slice is SBUF-resident with all chunk DMAs
    issued up front on the sync queue in consumption order (sync and
    scalar engine DMAs share one hardware ring, so a second "queue"
    only scrambles arrival order; gpsimd/Q0 is the only independent
    ring and carries wq + outputs). Critical small constants go FIRST
    on the sync queue -- on a side queue they starve behind the
    stream, and w/xT gate the PE start, whose lag back-pressures
    product-buffer recycling. The DVE has no setup work ahead of its
    first product mult. Measured ~65 us vs the 125.7 us baseline.
"""

import numpy as np

from contextlib import ExitStack

import concourse.bass as bass
import concourse.mybir as mybir
import concourse.tile as tile
from concourse.bass_utils import run_bass_kernel_spmd

F32 = mybir.dt.float32
F16 = mybir.dt.float16
U8 = mybir.dt.uint8
COPY = mybir.ActivationFunctionType.Copy

N_CORES = 8
BS, IN_F, OUT_F = 64, 1024, 1024
P = 128  # SBUF partitions; also the per-core contraction slice
HF = 512  # one fp32 psum bank
M = 8  # samples per matmul / group
SUB = P // M  # contraction sub-slice per sample within a matmul
NG = BS // M  # groups
NT = M  # t-tiles per group (one per contraction sub-slice)
GF = M * OUT_F  # free size of one group's noise tile
PROD_BUFS = 4


def _split_multi_waits(nc: bass.Bass) -> None:
    """walrus codegen on this toolchain accepts at most ONE sync-wait per
    instruction. Tile emits joins with several waits; hoist all but the last
    onto standalone EventSemaphore instructions (what wait_ge lowers to)
    immediately before the instruction, on the same engine stream."""
    for func in nc.m.functions:
        for block in func.blocks:
            out = []
            changed = False
            for inst in block.instructions:
                si = inst.sync_info
                if si is not None and si.on_wait and len(si.on_wait) > 1:
                    waits = list(si.on_wait)
                    for k, w in enumerate(waits[:-1]):
                        ev = mybir.InstEventSemaphore(
                            name=f"{inst.name}-sw{k}",
                            engine=inst.engine,
                            sync_info=mybir.SyncInfo(on_wait=[w], on_update=[]),
                        )
                        nc.register_instruction(ev)
                        out.append(ev)
                    inst.sync_info = mybir.SyncInfo(
                        on_wait=[waits[-1]], on_update=list(si.on_update or [])
                    )
                    changed = True
                out.append(inst)
            if changed:
                block.instructions = out


def build_bass(wq_scale: float = 1.0 / 255.0) -> bass.Bass:
    nc = bass.Bass(trn_type="TRN2", target_bir_lowering=False, debug=False)

    xT_d = nc.dram_tensor("xt16", [P, BS], F16, kind="ExternalInput").ap()
    w_d = nc.dram_tensor("w16", [P, OUT_F], F16, kind="ExternalInput").ap()
    wq_d = nc.dram_tensor("wqrep8", [P, GF], U8, kind="ExternalInput").ap()
    xb_d = nc.dram_tensor("xblk", [P, NG * NT * M], F16, kind="ExternalInput").ap()
    n_d = nc.dram_tensor("noise", [NG, P, GF], F16, kind="ExternalInput").ap()
    o_d = nc.dram_tensor("out", [BS, OUT_F], F16, kind="ExternalOutput").ap()

    with tile.TileContext(nc) as tc, ExitStack() as ctx:
        const = ctx.enter_context(tc.tile_pool(name="const", bufs=1))
        psump = ctx.enter_context(tc.tile_pool(name="psum", bufs=2, space="PSUM"))
        prodp = ctx.enter_context(tc.tile_pool(name="prod", bufs=PROD_BUFS))
        outp = ctx.enter_context(tc.tile_pool(name="outp", bufs=2))

        # Small constants FIRST on the sync queue, ahead of its noise
        # chunks: w/xT gate the PE start (the xw seed matmuls lead each
        # group in PE program order), and a late PE start stalls product
        # buffer recycling and slides the whole pipeline right.
        HB = GF // 2
        wq8 = const.tile([P, GF], U8)
        nc.sync.dma_start(wq8[:, :HB], wq_d[:, :HB])
        w_h = const.tile([P, OUT_F], F16)
        nc.sync.dma_start(w_h[:], w_d)
        xT_h = const.tile([P, BS], F16)
        nc.sync.dma_start(xT_h[:], xT_d)
        xblk = const.tile([P, NG * NT * M], F16)
        nc.sync.dma_start(xblk[:], xb_d)
        nc.sync.dma_start(wq8[:, HB:], wq_d[:, HB:])

        # |w| pre-replicated into the interleaved (j,u) x (t,o) layout,
        # quantized to uint8 (1 MB instead of 2 MB f16 -- the ~435 GB/s
        # DMA-engine aggregate is the kernel's pole, so bytes are the
        # only lever). wq8 rides the sync queue head too: gpsimd stays
        # completely unused so its queue setup and end-of-kernel drain
        # disappear. The scalar engine dequantizes to f16 once.
        # (Quarter-granular wq DMA + dequant measured ~3 us WORSE.)
        wq_sb = const.tile([P, GF], F16)
        for hh in range(2):
            nc.scalar.activation(
                wq_sb[:, hh * HB : (hh + 1) * HB],
                wq8[:, hh * HB : (hh + 1) * HB],
                COPY,
                scale=wq_scale,
            )

        # The whole 16 MB noise slice is SBUF-resident (fits the 208
        # KB/partition budget) and all chunk DMAs issue up front on the
        # sync queue in consumption order. (sync and scalar engine DMAs
        # share ONE hardware ring -- interleaving them scrambles arrival
        # order away from consumption order; measured 12 us worse.)
        # No buffer recycling -> the stream never stalls on compute.
        NCHUNK = 2 * NG
        noise_sb = const.tile([P, NCHUNK * HB], F16)
        for ci in range(NCHUNK):
            nc.sync.dma_start(
                noise_sb[:, ci * HB : (ci + 1) * HB],
                n_d[ci // 2][:, (ci % 2) * HB : (ci % 2 + 1) * HB],
            )

        for g in range(NG):
            s0 = g * M
            # x@w seeds this group's psum blocks (batched f16 matmul).
            pss = []
            for h in range(2):
                ps = psump.tile([M, HF], F32, name=f"ps{g}_{h}", tag=f"ps{h}")
                nc.tensor.matmul(
                    ps[:, :],
                    lhsT=xT_h[:, s0 : s0 + M],
                    rhs=w_h[:, h * HF : (h + 1) * HF],
                    start=True,
                    stop=False,
                    skip_group_check=True,
                )
                pss.append(ps)
            # Mult granularity decouples from DMA chunking (noise is
            # resident): last group runs quarter mults for a finer
            # DVE->PE drain tail (all-quarters measured slightly worse:
            # per-instruction overhead outweighs the tail gain).
            nch = 4 if g == NG - 1 else 2
            tpc = NT // nch
            cf = GF // nch
            for c in range(nch):
                pt = prodp.tile([P, cf], F16, name="pt", tag="pt")
                nc.vector.tensor_tensor(
                    pt[:],
                    noise_sb[:, g * GF + c * cf : g * GF + (c + 1) * cf],
                    wq_sb[:, c * cf : (c + 1) * cf],
                    mybir.AluOpType.mult,
                )
                for tl in range(tpc):
                    t = c * tpc + tl
                    for h in range(2):
                        nc.tensor.matmul(
                            pss[h][:, :],
                            lhsT=xblk[:, (g * NT + t) * M : (g * NT + t + 1) * M],
                            rhs=pt[:, tl * OUT_F + h * HF : tl * OUT_F + h * HF + HF],
                            start=False,
                            stop=(t == NT - 1),
                            skip_group_check=True,
                        )
            # f16 output (partials ~O(1); host re-sums in f64). GPSIMD
            # cannot access PSUM, so the copies run on the scalar engine
            # (all its dma_starts were already issued above).
            out_sb = outp.tile([M, OUT_F], F16, name="osb", tag="osb")
            for h in range(2):
                nc.scalar.activation(
                    out_sb[:, h * HF : (h + 1) * HF], pss[h][:, :], COPY
                )
            # out DMA issues on the scalar engine right after its copies
            # (natural program order; Act is otherwise idle there).
            nc.scalar.dma_start(o_d[s0 : s0 + M, :], out_sb[:])

    _split_multi_waits(nc)
    return nc


def wq_scale_for(weight) -> float:
    return float(np.abs(weight).max()) / 255.0


def make_in_maps(x, weight, bias, noise):
    x = np.ascontiguousarray(x, dtype=np.float32)
    weight = np.ascontiguousarray(weight, dtype=np.float32)
    scale = wq_scale_for(weight)
    in_maps = []
    for k in range(N_CORES):
        sl = slice(k * P, (k + 1) * P)
        w_k = weight[sl, :]  # [P, OUT_F]
        x_k = x[:, sl]  # [BS, P]

        # noise interleave: partition j*SUB+u <- sample g*M+j, i-row t*SUB+u,
        # free dim ordered (t, o).
        nv = np.ascontiguousarray(noise[:, sl, :], dtype=np.float32)
        nv = nv.reshape(NG, M, NT, SUB, OUT_F)  # [g, j, t, u, o]
        nv = nv.transpose(0, 1, 3, 2, 4).astype(np.float16)  # [g, j, u, t, o]
        nv = np.ascontiguousarray(nv).reshape(NG, P, GF)

        # |w| replicated over j in the same layout, uint8-quantized with
        # a global scale (dequantized once on device).
        wq = np.abs(w_k).reshape(NT, SUB, OUT_F).transpose(1, 0, 2)  # [u, t, o]
        wq = np.rint(wq / scale).astype(np.uint8)
        wq = np.broadcast_to(wq[None], (M, SUB, NT, OUT_F))
        wq = np.ascontiguousarray(wq).reshape(P, GF)

        # Block-diagonal x: xblk[j*SUB+u, ((g*NT+t)*M)+m] =
        #   x[g*M+m, t*SUB+u] if j == m else 0.
        xb = np.zeros((M, SUB, NG, NT, M), dtype=np.float16)
        xr = x_k.reshape(NG, M, NT, SUB)  # [g, j, t, u]
        for j in range(M):
            xb[j, :, :, :, j] = xr[:, j].transpose(2, 0, 1)  # [u, g, t]
        xb = xb.reshape(P, NG * NT * M)

        in_maps.append(
            {
                "xt16": np.ascontiguousarray(x_k.T).astype(np.float16),
                "w16": w_k.astype(np.float16),
                "wqrep8": wq,
                "xblk": np.ascontiguousarray(xb),
                "noise": nv,
            }
        )
    return in_maps


def assemble(results, bias) -> np.ndarray:
    acc = np.zeros((BS, OUT_F), dtype=np.float64)
    for k in range(N_CORES):
        acc += results[k]["out"].astype(np.float64)
    acc += np.asarray(bias, dtype=np.float64)[None, :]
    return acc.astype(np.float32)


def kernel(**inputs) -> np.ndarray:
    nc = build_bass(wq_scale_for(inputs["weight"]))
    in_maps = make_in_maps(
        inputs["x"], inputs["weight"], inputs["bias"], inputs["noise"]
    )
    res = run_bass_kernel_spmd(nc, in_maps, core_ids=list(range(N_CORES)))
    return assemble(res.results, inputs["bias"])


if __name__ == "__main__":
    rng = np.random.default_rng(0)
    x = rng.standard_normal((BS, IN_F), dtype=np.float32)
    w = rng.standard_normal((IN_F, OUT_F), dtype=np.float32) * 0.03
    b = rng.standard_normal((OUT_F,), dtype=np.float32) * 0.03
    s = (rng.random((BS, IN_F, OUT_F)) < 0.5).astype(np.float32) * 2 - 1
    out = kernel(x=x, weight=w, bias=b, noise=s)
    ref = np.einsum("bi,bio->bo", x, w[None] + np.abs(w)[None] * s) + b
    err = np.abs(out - ref).max() / np.abs(ref).max()
    print("rel err:", err)


# revision 47
# speedup vs baseline: 1.0288x; 1.0288x over previous
"""Bridgeout FC layer (dense_mlp) Trainium2 kernel.

out[b, o] = sum_i x[b,i] * (w[i,o] + |w[i,o]| * noise[b,i,o]) + bias[o]

Strategy (8 NeuronCores, contraction-parallel):
  - Each core owns a 128-row slice of the contraction index i. It reads
    noise[:, islice, :] (its 32 MB share of the 256 MB noise tensor) and
    weight[islice, :] (NOT replicated as batch sharding would) and
    produces partial[b, o] = sum_{i in islice} x*(w+|w|*noise); the
    host adds the 8 partials plus the bias.
  - Everything ships as float16 (the 2e-2 rel-err gate leaves >10x
    margin over fp16's 0.05% element error), halving noise DMA bytes:
    16 MB/core at the measured ~420-440 GB/s aggregate DMA rate.
  - Block-diagonal matmuls: the i-slice is split into 8 sub-slices of
    16; the host interleaves noise so SBUF partition j*16+u holds
    sample (g*8+j)'s sub-row u. Then lhsT[128, 8] is a block-diagonal
    x matrix (zero blocks kill cross-sample terms) and ONE matmul
    computes a [8 samples, 512] psum block while streaming 512 f16
    columns at 1 col/cycle -- wide on both M and N. Naive alternatives
    lose: per-sample moving-product matmuls (M=1, N=512) leave 64
    narrow [1,512] psum tiles whose copies serialize ~55 us on the
    scalar engine (matmul psum writes must start at partition 0/32/64,
    so rows can't pack); per-sample stationary-product matmuls (M=128,
    N=1) pay ~170 ns pipeline+LDWEIGHTS cost per single streamed
    column (measured: 79 us kernel, PE-bound).
  - The product pt = |w| (*) noise runs on the DVE in f16 (16-bit
    packed operands hit the 2x mode; measured 2.29 us per [128,4096]
    chunk ~ matching the 2.38 us DMA cadence of 1 MB chunks). |w|
    ships pre-replicated in the interleaved layout, quantized to uint8
    (1 MB; the ~435 GB/s 16-engine DMA aggregate is the kernel's pole,
    so bytes are the only lever) and is dequantized once on the scalar
    engine with a global scale baked into the program.
  - The x@w term seeds each group's psum block with one batched f16
    matmul (lhsT = xT columns) before the noise matmuls accumulate.
  - The whole 16 MB noise slice is SBUF-resident with all chunk DMAs
    issued up front on the sync queue in consumption order (sync and
    scalar engine DMAs share one hardware ring, so a second "queue"
    only scrambles arrival order; gpsimd/Q0 is the only independent
    ring and carries wq + outputs). Critical small constants go FIRST
    on the sync queue -- on a side queue they starve behind the
    stream, and w/xT gate the PE start, whose lag back-pressures
    product-buffer recycling. The DVE has no setup work ahead of its
    first product mult. Measured ~65 us vs the 125.7 us baseline.
"""

import numpy as np

from contextlib import ExitStack

import concourse.bass as bass
import concourse.mybir as mybir
import concourse.tile as tile
from concourse.bass_utils import run_bass_kernel_spmd

F32 = mybir.dt.float32
F16 = mybir.dt.float16
U8 = mybir.dt.uint8
COPY = mybir.ActivationFunctionType.Copy

N_CORES = 8
BS, IN_F, OUT_F = 64, 1024, 1024
P = 128  # SBUF partitions; also the per-core contraction slice
HF = 512  # one fp32 psum bank
M = 8  # samples per matmul / group
SUB = P // M  # contraction sub-slice per sample within a matmul
NG = BS // M  # groups
NT = M  # t-tiles per group (one per contraction sub-slice)
GF = M * OUT_F  # free size of one group's noise tile
PROD_BUFS = 4


def _split_multi_waits(nc: bass.Bass) -> None:
    """walrus codegen on this toolchain accepts at most ONE sync-wait per
    instruction. Tile emits joins with several waits; hoist all but the last
    onto standalone EventSemaphore instructions (what wait_ge lowers to)
    immediately before the instruction, on the same engine stream."""
    for func in nc.m.functions:
        for block in func.blocks:
            out = []
            changed = False
            for inst in block.instructions:
                si = inst.sync_info
                if si is not None and si.on_wait and len(si.on_wait) > 1:
                    waits = list(si.on_wait)
                    for k, w in enumerate(waits[:-1]):
                        ev = mybir.InstEventSemaphore(
                            name=f"{inst.name}-sw{k}",
                            engine=inst.engine,
                            sync_info=mybir.SyncInfo(on_wait=[w], on_update=[]),
                        )
                        nc.register_instruction(ev)
                        out.append(ev)
                    inst.sync_info = mybir.SyncInfo(
                        on_wait=[waits[-1]], on_update=list(si.on_update or [])
                    )
                    changed = True
                out.append(inst)
            if changed:
                block.instructions = out


def build_bass(wq_scale: float = 1.0 / 255.0) -> bass.Bass:
    nc = bass.Bass(trn_type="TRN2", target_bir_lowering=False, debug=False)

    xT_d = nc.dram_tensor("xt16", [P, BS], F16, kind="ExternalInput").ap()
    w_d = nc.dram_tensor("w16", [P, OUT_F], F16, kind="ExternalInput").ap()
    wq_d = nc.dram_tensor("wqrep8", [P, GF], U8, kind="ExternalInput").ap()
    xb_d = nc.dram_tensor("xblk", [P, NG * NT * M], F16, kind="ExternalInput").ap()
    n_d = nc.dram_tensor("noise", [NG, P, GF], F16, kind="ExternalInput").ap()
    o_d = nc.dram_tensor("out", [BS, OUT_F], F16, kind="ExternalOutput").ap()

    with tile.TileContext(nc) as tc, ExitStack() as ctx:
        const = ctx.enter_context(tc.tile_pool(name="const", bufs=1))
        psump = ctx.enter_context(tc.tile_pool(name="psum", bufs=2, space="PSUM"))
        prodp = ctx.enter_context(tc.tile_pool(name="prod", bufs=PROD_BUFS))
        outp = ctx.enter_context(tc.tile_pool(name="outp", bufs=2))

        # Small constants FIRST on the sync queue, ahead of its noise
        # chunks: w/xT gate the PE start (the xw seed matmuls lead each
        # group in PE program order), and a late PE start stalls product
        # buffer recycling and slides the whole pipeline right.
        HB = GF // 2
        w_h = const.tile([P, OUT_F], F16)
        nc.sync.dma_start(w_h[:], w_d)
        xT_h = const.tile([P, BS], F16)
        nc.sync.dma_start(xT_h[:], xT_d)
        xblk = const.tile([P, NG * NT * M], F16)
        nc.sync.dma_start(xblk[:], xb_d)

        # |w| pre-replicated into the interleaved (j,u) x (t,o) layout,
        # quantized to uint8 (1 MB instead of 2 MB f16 -- the ~435 GB/s
        # DMA-engine aggregate is the kernel's pole, so bytes are the
        # only lever). Halves ride the gpsimd queue: its SWDGE path adds
        # capacity alongside the main ring (moving them onto sync
        # measured 5.7 us worse). The scalar engine dequantizes once.
        # (Quarter-granular wq DMA + dequant measured ~3 us WORSE.)
        wq8 = const.tile([P, GF], U8)
        nc.gpsimd.dma_start(wq8[:, :HB], wq_d[:, :HB])
        nc.gpsimd.dma_start(wq8[:, HB:], wq_d[:, HB:])
        wq_sb = const.tile([P, GF], F16)
        for hh in range(2):
            nc.scalar.activation(
                wq_sb[:, hh * HB : (hh + 1) * HB],
                wq8[:, hh * HB : (hh + 1) * HB],
                COPY,
                scale=wq_scale,
            )

        # The whole 16 MB noise slice is SBUF-resident (fits the 208
        # KB/partition budget) and all chunk DMAs issue up front on the
        # sync queue in consumption order. (sync and scalar engine DMAs
        # share ONE hardware ring -- interleaving them scrambles arrival
        # order away from consumption order; measured 12 us worse.)
        # No buffer recycling -> the stream never stalls on compute.
        # 2 MB full-group transfers (16 KB partition lines, half the
        # descriptors per byte) except the last group, which lands as
        # halves so the drain tail starts sooner.
        NCHUNK = 2 * NG
        noise_sb = const.tile([P, NCHUNK * HB], F16)
        for g0 in range(NG):
            if g0 < NG - 1:
                nc.sync.dma_start(
                    noise_sb[:, g0 * GF : (g0 + 1) * GF], n_d[g0][:, :]
                )
            else:
                for half in range(2):
                    nc.sync.dma_start(
                        noise_sb[:, g0 * GF + half * HB : g0 * GF + (half + 1) * HB],
                        n_d[g0][:, half * HB : (half + 1) * HB],
                    )

        for g in range(NG):
            s0 = g * M
            # x@w seeds this group's psum blocks (batched f16 matmul).
            pss = []
            for h in range(2):
                ps = psump.tile([M, HF], F32, name=f"ps{g}_{h}", tag=f"ps{h}")
                nc.tensor.matmul(
                    ps[:, :],
                    lhsT=xT_h[:, s0 : s0 + M],
                    rhs=w_h[:, h * HF : (h + 1) * HF],
                    start=True,
                    stop=False,
                    skip_group_check=True,
                )
                pss.append(ps)
            # Mult granularity decouples from DMA chunking (noise is
            # resident): last group runs quarter mults for a finer
            # DVE->PE drain tail (all-quarters measured slightly worse:
            # per-instruction overhead outweighs the tail gain).
            nch = 4 if g == NG - 1 else 2
            tpc = NT // nch
            cf = GF // nch
            for c in range(nch):
                pt = prodp.tile([P, cf], F16, name="pt", tag="pt")
                nc.vector.tensor_tensor(
                    pt[:],
                    noise_sb[:, g * GF + c * cf : g * GF + (c + 1) * cf],
                    wq_sb[:, c * cf : (c + 1) * cf],
                    mybir.AluOpType.mult,
                )
                for tl in range(tpc):
                    t = c * tpc + tl
                    for h in range(2):
                        nc.tensor.matmul(
                            pss[h][:, :],
                            lhsT=xblk[:, (g * NT + t) * M : (g * NT + t + 1) * M],
                            rhs=pt[:, tl * OUT_F + h * HF : tl * OUT_F + h * HF + HF],
                            start=False,
                            stop=(t == NT - 1),
                            skip_group_check=True,
                        )
            # f16 output (partials ~O(1); host re-sums in f64). GPSIMD
            # cannot access PSUM, so the copies run on the scalar engine
            # (all its dma_starts were already issued above).
            out_sb = outp.tile([M, OUT_F], F16, name="osb", tag="osb")
            for h in range(2):
                nc.scalar.activation(
                    out_sb[:, h * HF : (h + 1) * HF], pss[h][:, :], COPY
                )
            nc.gpsimd.dma_start(o_d[s0 : s0 + M, :], out_sb[:])

    _split_multi_waits(nc)
    return nc


def wq_scale_for(weight) -> float:
    return float(np.abs(weight).max()) / 255.0


def make_in_maps(x, weight, bias, noise):
    x = np.ascontiguousarray(x, dtype=np.float32)
    weight = np.ascontiguousarray(weight, dtype=np.float32)
    scale = wq_scale_for(weight)
    in_maps = []
    for k in range(N_CORES):
        sl = slice(k * P, (k + 1) * P)
        w_k = weight[sl, :]  # [P, OUT_F]
        x_k = x[:, sl]  # [BS, P]

        # noise interleave: partition j*SUB+u <- sample g*M+j, i-row t*SUB+u,
        # free dim ordered (t, o).
        nv = np.ascontiguousarray(noise[:, sl, :], dtype=np.float32)
        nv = nv.reshape(NG, M, NT, SUB, OUT_F)  # [g, j, t, u, o]
        nv = nv.transpose(0, 1, 3, 2, 4).astype(np.float16)  # [g, j, u, t, o]
        nv = np.ascontiguousarray(nv).reshape(NG, P, GF)

        # |w| replicated over j in the same layout, uint8-quantized with
        # a global scale (dequantized once on device).
        wq = np.abs(w_k).reshape(NT, SUB, OUT_F).transpose(1, 0, 2)  # [u, t, o]
        wq = np.rint(wq / scale).astype(np.uint8)
        wq = np.broadcast_to(wq[None], (M, SUB, NT, OUT_F))
        wq = np.ascontiguousarray(wq).reshape(P, GF)

        # Block-diagonal x: xblk[j*SUB+u, ((g*NT+t)*M)+m] =
        #   x[g*M+m, t*SUB+u] if j == m else 0.
        xb = np.zeros((M, SUB, NG, NT, M), dtype=np.float16)
        xr = x_k.reshape(NG, M, NT, SUB)  # [g, j, t, u]
        for j in range(M):
            xb[j, :, :, :, j] = xr[:, j].transpose(2, 0, 1)  # [u, g, t]
        xb = xb.reshape(P, NG * NT * M)

        in_maps.append(
            {
                "xt16": np.ascontiguousarray(x_k.T).astype(np.float16),
                "w16": w_k.astype(np.float16),
                "wqrep8": wq,
                "xblk": np.ascontiguousarray(xb),
                "noise": nv,
            }
        )
    return in_maps


def assemble(results, bias) -> np.ndarray:
    acc = np.zeros((BS, OUT_F), dtype=np.float64)
    for k in range(N_CORES):
        acc += results[k]["out"].astype(np.float64)
    acc += np.asarray(bias, dtype=np.float64)[None, :]
    return acc.astype(np.float32)


def kernel(**inputs) -> np.ndarray:
    nc = build_bass(wq_scale_for(inputs["weight"]))
    in_maps = make_in_maps(
        inputs["x"], inputs["weight"], inputs["bias"], inputs["noise"]
    )
    res = run_bass_kernel_spmd(nc, in_maps, core_ids=list(range(N_CORES)))
    return assemble(res.results, inputs["bias"])


if __name__ == "__main__":
    rng = np.random.default_rng(0)
    x = rng.standard_normal((BS, IN_F), dtype=np.float32)
    w = rng.standard_normal((IN_F, OUT_F), dtype=np.float32) * 0.03
    b = rng.standard_normal((OUT_F,), dtype=np.float32) * 0.03
    s = (rng.random((BS, IN_F, OUT_F)) < 0.5).astype(np.float32) * 2 - 1
    out = kernel(x=x, weight=w, bias=b, noise=s)
    ref = np.einsum("bi,bio->bo", x, w[None] + np.abs(w)[None] * s) + b
    err = np.abs(out - ref).max() / np.abs(ref).max()
    print("rel err:", err)
